# revision 34
# baseline (speedup 1.0000x reference)
"""Trainium2 Bass kernel for BConvAttention2d.

Per core: 4 images as 2 pairs (2 images x 64ch -> 128 partitions).
Per (pair, patch-row group g of 8 patches):
  1. DMA 16 image rows f32; ACT sign -> bf16 into patch-padded binp_g
     [128, 8p, 18, 18] (borders pre-zeroed).
  2. DVE: 9 broadcast multiplies tmp_t = binp_g * w_t[c,p] (bf16 2x mode),
     written into padded tmp tiles (borders stay zero).
  3. PE: depthwise tap accumulation as identity-matmuls with shifted-window
     rhs APs: psum_g += I.T @ tmp_t[:, :, u:u+16, v:v+16].  Patch-local
     padding means shifts never cross patches.
  4. ACT: Sign(psum_g) -> bsa (image-padded bf16). HW Sign(0)=0 matches jnp.
  5. PE: dense 3x3 conv 64->64ch: 9 accumulating bf16 matmuls per 4-row
     chunk, block-diag weights pack both images; ACT evicts PSUM->SBUF f32;
     DMA out.

All values +-1/0 -> bf16 inputs + f32 PSUM accumulation are exact.
Filters are tiny: sign + layout repack happens on host.
"""

import numpy as np
import ml_dtypes

import concourse.bass as bass
import concourse.mybir as mybir
from concourse.tile import TileContext
from concourse.bass_utils import run_bass_kernel_spmd

# ---- problem constants (hardcoded per contract) ----
B, C, H, W = 32, 64, 128, 128
N_CORES = 8
B_CORE = B // N_CORES          # 4 images per core
N_PAIRS = B_CORE // 2
PATCH = 16
NP_SIDE = H // PATCH           # 8x8 patch grid
NPATCH = NP_SIDE * NP_SIDE
TAPS = 9
# depthwise taps whose accumulation runs on DVE instead of PE identity-matmuls
DVE_TAPS = (0, 2, 6)
BF16 = mybir.dt.bfloat16
F32 = mybir.dt.float32

_CACHED_NC = None


def _split_multiwaits(nc):
    """walrus codegen in this toolchain accepts only ONE embedded sync wait
    per instruction; hoist extras onto preceding NOPs on the same engine."""
    for f in nc.m.functions:
        for blk in f.blocks:
            new_insts = []
            for inst in blk.instructions:
                si = inst.sync_info
                if si is not None and len(si.on_wait) > 1:
                    waits = list(si.on_wait)
                    for w in waits[:-1]:
                        nop = mybir.InstNoOp(
                            name=nc.get_next_instruction_name(), ins=[], outs=[]
                        )
                        nop.engine = inst.engine
                        nop.sync_info = mybir.SyncInfo(on_wait=[w], on_update=[])
                        new_insts.append(nop)
                    inst.sync_info = mybir.SyncInfo(
                        on_wait=[waits[-1]], on_update=list(si.on_update)
                    )
                new_insts.append(inst)
            blk.instructions[:] = new_insts


def _build_nc():
    nc = bass.Bass()
    x = nc.declare_dram_parameter("x", [B_CORE, C, H, W], F32, isOutput=False)
    wblk = nc.declare_dram_parameter("wblk", [128, TAPS, 128], BF16, isOutput=False)
    wbm = nc.declare_dram_parameter(
        "wbm", [128, TAPS, NPATCH, PATCH], BF16, isOutput=False
    )
    wid = nc.declare_dram_parameter("wid", [128, 128], BF16, isOutput=False)
    y = nc.declare_dram_parameter("y", [B_CORE, C, H, W], F32, isOutput=True)

    with TileContext(nc) as tc:
        with (
            tc.tile_pool(name="persist", bufs=1) as persist,
            tc.tile_pool(name="binpp", bufs=1) as binpool,
            tc.tile_pool(name="tmpp", bufs=1) as tmppool,
            tc.tile_pool(name="inp", bufs=3) as inpool,
            tc.tile_pool(name="outp", bufs=6) as outpool,
            tc.tile_pool(name="dwps", bufs=2, space="PSUM") as dwpsum,
            tc.tile_pool(name="cvps", bufs=3, space="PSUM") as cvpsum,
        ):
            wblk_sb = persist.tile([128, TAPS, 128], BF16)
            wb_sb = persist.tile([128, TAPS, NPATCH, PATCH], BF16)
            wid_sb = persist.tile([128, 128], BF16)
            bsa0 = persist.tile([128, H + 2, W + 2], BF16)
            bsa1 = persist.tile([128, H + 2, W + 2], BF16)

            nc.sync.dma_start(out=wid_sb, in_=wid[:])
            nc.sync.dma_start(out=wblk_sb, in_=wblk[:])
            nc.sync.dma_start(out=wb_sb, in_=wbm[:])

            def zero_borders_patch(t):
                # pad ring of each 18x18 patch block; interior rewritten later
                nc.gpsimd.memset(t[:, :, 0, :], 0.0)
                nc.gpsimd.memset(t[:, :, 17, :], 0.0)
                nc.gpsimd.memset(t[:, :, 1:17, 0], 0.0)
                nc.gpsimd.memset(t[:, :, 1:17, 17], 0.0)

            # pre-zero the padded pool tiles once (interiors get rewritten,
            # borders must remain zero forever); zero in first-use order so
            # group 0's tiles clear the GPSIMD queue first
            binp_tiles = [
        binpool.tile([128, NP_SIDE, PATCH + 2, PATCH + 2], BF16, name=f"binp{i}")
        for i in range(4)
            ]
            tmp_tiles = [
        tmppool.tile([128, NP_SIDE, PATCH + 2, PATCH + 2], BF16, name=f"tmp{i}")
        for i in range(6)
            ]
            part_tiles = [
        tmppool.tile([128, NP_SIDE, PATCH + 2, PATCH + 2], BF16, name=f"part{i}")
        for i in range(2)
            ]
            zero_borders_patch(binp_tiles[0])
            for tt in tmp_tiles:
                zero_borders_patch(tt)
            zero_borders_patch(part_tiles[0])
            zero_borders_patch(part_tiles[1])
            for bt in binp_tiles[1:]:
                zero_borders_patch(bt)
            # bsa borders are not needed until the first sign2 (~30us in), so
            # they go last in the GPSIMD queue
            for bs in (bsa0, bsa1):
                nc.gpsimd.memset(bs[:, 0, :], 0.0)
                nc.gpsimd.memset(bs[:, 129, :], 0.0)
                nc.gpsimd.memset(bs[:, 1:129, 0], 0.0)
                nc.gpsimd.memset(bs[:, 1:129, 129], 0.0)
            scr_tile = tmppool.tile(
                [128, NP_SIDE, PATCH, PATCH], BF16, name="scr"
            )

            def conv_group(y_pair, bsa, g):
                # conv output rows 16g..16g+15 in chunks of 4 rows
                for cc in range(4):
                    y0 = 16 * g + 4 * cc
                    pt = cvpsum.tile([128, 4, W], F32, name="cvp")
                    for t in range(TAPS):
                        u, v = divmod(t, 3)
                        nc.tensor.matmul(
                            pt,
                            lhsT=wblk_sb[:, t, :],
                            rhs=bsa[:, y0 + u : y0 + u + 4, v : v + W],
                            start=(t == 0),
                            stop=(t == TAPS - 1),
                        )
                    ot = outpool.tile([128, 4, W], F32, name="ot")
                    nc.scalar.activation(
                        ot, pt, mybir.ActivationFunctionType.Copy
                    )
                    nc.sync.dma_start(
                        out=y_pair[:, :, y0 : y0 + 4, :].rearrange(
                            "b c h w -> (b c) h w"
                        ),
                        in_=ot,
                    )

            bi = 0
            ti = 0
            for pi in range(N_PAIRS):
                x_pair = x[2 * pi : 2 * pi + 2]
                y_pair = y[2 * pi : 2 * pi + 2]
                bsa = bsa0 if pi % 2 == 0 else bsa1

                for g in range(NP_SIDE):
                    # ---- load + sign(input) for patch-row g ----
                    ch = inpool.tile([128, PATCH * W], F32, name="ch")
                    nc.sync.dma_start(
                        out=ch,
                        in_=x_pair[:, :, PATCH * g : PATCH * (g + 1), :].rearrange(
                            "b c h w -> (b c) (h w)"
                        ),
                    )
                    binp = binp_tiles[bi % 4]
                    bi += 1
                    nc.scalar.sign(
                        out=binp[:, :, 1:17, 1:17],
                        in_=ch.rearrange(
                            "n (i pc j) -> n pc i j", pc=NP_SIDE, j=PATCH
                        ),
                    )

                    # ---- depthwise: DVE weighted copies + PE tap-sum ----
                    # DVE_TAPS are pre-summed on DVE (shifted-window mults +
                    # adds into a padded partial tile read back through the
                    # center window); the rest go through identity-matmuls.
                    wbr = lambda t: wb_sb[
                        :, t, 8 * g : 8 * g + 8, None, :
                    ].broadcast_to([128, NP_SIDE, PATCH, PATCH])
                    # taper: pair 1's last groups shift more tap-sums onto
                    # DVE, whose work otherwise ends ~23us before PE's
                    dset = DVE_TAPS
                    if pi == N_PAIRS - 1 and g >= 5:
                        dset = (0, 2, 5, 6, 8)
                    part = part_tiles[(pi * NP_SIDE + g) % 2]
                    scr = scr_tile
                    for i, t in enumerate(dset):
                        u, v = divmod(t, 3)
                        win = binp[:, :, u : u + PATCH, v : v + PATCH]
                        if i == 0:
                            nc.vector.tensor_tensor(
                                out=part[:, :, 1:17, 1:17], in0=win, in1=wbr(t),
                                op=mybir.AluOpType.mult,
                            )
                        else:
                            nc.vector.tensor_tensor(
                                out=scr, in0=win, in1=wbr(t),
                                op=mybir.AluOpType.mult,
                            )
                            nc.vector.tensor_add(
                                out=part[:, :, 1:17, 1:17],
                                in0=part[:, :, 1:17, 1:17],
                                in1=scr,
                            )
                    pe_items = [(t, t) for t in range(TAPS) if t not in dset]
                    if dset:
                        pe_items.append((4, None))  # partial via center window
                    pe_tmps = []
                    for t, tap in pe_items:
                        if tap is not None:
                            tmp = tmp_tiles[ti % 6]
                            ti += 1
                            nc.vector.tensor_tensor(
                                out=tmp[:, :, 1:17, 1:17],
                                in0=binp[:, :, 1:17, 1:17],
                                in1=wbr(tap),
                                op=mybir.AluOpType.mult,
                            )
                        else:
                            tmp = part
                        pe_tmps.append(tmp)
                    # half-group psum tiles so sign2 overlaps the next half's MMs
                    for hg in range(2):
                        pg = dwpsum.tile([128, 4, PATCH, PATCH], F32, name="pg")
                        for idx, (t, tap) in enumerate(pe_items):
                            u, v = divmod(t, 3)
                            tmp = pe_tmps[idx]
                            for pi2 in range(2):
                                pp = 4 * hg + 2 * pi2
                                nc.tensor.matmul(
                                    pg[:, 2 * pi2 : 2 * pi2 + 2],
                                    lhsT=wid_sb,
                                    rhs=tmp[
                                        :, pp : pp + 2, u : u + PATCH, v : v + PATCH
                                    ],
                                    start=(idx == 0),
                                    stop=(idx == len(pe_items) - 1),
                                )
                        # ---- sign(self_attention) -> image-padded bsa ----
                        nc.scalar.sign(
                            out=bsa[
                                :,
                                1 + PATCH * g : 1 + PATCH * (g + 1),
                                1 + 64 * hg : 65 + 64 * hg,
                            ].rearrange("n i (pc j) -> n pc i j", pc=4),
                            in_=pg,
                        )

                    # ---- final conv lags one group (needs bsa row 16g+16) ----
                    if g >= 1:
                        conv_group(y_pair, bsa, g - 1)
                conv_group(y_pair, bsa, NP_SIDE - 1)

    _split_multiwaits(nc)
    return nc


def _host_weights(patch_filters, output_filters):
    # block-diagonal lhsT for the final conv: [tap, k=cin+64*s, m=cout+64*s]
    ofs = np.sign(np.asarray(output_filters, np.float32))          # [o, i, 3, 3]
    oft = ofs.transpose(2, 3, 1, 0).reshape(TAPS, C, C)            # [t, cin, cout]
    wblk = np.zeros((TAPS, 128, 128), np.float32)
    wblk[:, :C, :C] = oft
    wblk[:, C:, C:] = oft
    wblk = wblk.transpose(1, 0, 2)                                 # [k, t, m]
    # depthwise weights broadcast along j: [t, c + 64*s, p, j]
    pfs = np.sign(np.asarray(patch_filters, np.float32))[:, :, 0]  # [c, p, 3, 3]
    wb = pfs.transpose(2, 3, 0, 1).reshape(TAPS, C, NPATCH)        # [t, c, p]
    wbm = np.repeat(wb[:, :, :, None], PATCH, axis=3)              # [t, c, p, j]
    wbm = np.concatenate([wbm, wbm], axis=1)                       # [t, 128, p, j]
    wbm = wbm.transpose(1, 0, 2, 3)                                # [k, t, p, j]
    wid = np.eye(128, dtype=np.float32)
    to_bf = lambda a: np.ascontiguousarray(a).astype(ml_dtypes.bfloat16)
    return to_bf(wblk), to_bf(wbm), to_bf(wid)


def kernel(input, k, t, patch_filters, output_filters):
    global _CACHED_NC
    if _CACHED_NC is None:
        _CACHED_NC = _build_nc()
    nc = _CACHED_NC

    x = np.ascontiguousarray(np.asarray(input, np.float32))
    wblk, wbm, wid = _host_weights(patch_filters, output_filters)
    in_maps = [
        {"x": np.ascontiguousarray(x[i * B_CORE : (i + 1) * B_CORE]),
         "wblk": wblk, "wbm": wbm, "wid": wid}
        for i in range(N_CORES)
    ]
    res = run_bass_kernel_spmd(nc, in_maps, list(range(N_CORES)))
    return np.concatenate([r["y"] for r in res.results], axis=0)


# revision 35
# speedup vs baseline: 1.0037x; 1.0037x over previous
"""Trainium2 Bass kernel for BConvAttention2d.

Per core: 4 images as 2 pairs (2 images x 64ch -> 128 partitions).
Per (pair, patch-row group g of 8 patches):
  1. DMA 16 image rows f32; ACT sign -> bf16 into patch-padded binp_g
     [128, 8p, 18, 18] (borders pre-zeroed).
  2. DVE: 9 broadcast multiplies tmp_t = binp_g * w_t[c,p] (bf16 2x mode),
     written into padded tmp tiles (borders stay zero).
  3. PE: depthwise tap accumulation as identity-matmuls with shifted-window
     rhs APs: psum_g += I.T @ tmp_t[:, :, u:u+16, v:v+16].  Patch-local
     padding means shifts never cross patches.
  4. ACT: Sign(psum_g) -> bsa (image-padded bf16). HW Sign(0)=0 matches jnp.
  5. PE: dense 3x3 conv 64->64ch: 9 accumulating bf16 matmuls per 4-row
     chunk, block-diag weights pack both images; ACT evicts PSUM->SBUF f32;
     DMA out.

All values +-1/0 -> bf16 inputs + f32 PSUM accumulation are exact.
Filters are tiny: sign + layout repack happens on host.
"""

import numpy as np
import ml_dtypes

import concourse.bass as bass
import concourse.mybir as mybir
from concourse.tile import TileContext
from concourse.bass_utils import run_bass_kernel_spmd

# ---- problem constants (hardcoded per contract) ----
B, C, H, W = 32, 64, 128, 128
N_CORES = 8
B_CORE = B // N_CORES          # 4 images per core
N_PAIRS = B_CORE // 2
PATCH = 16
NP_SIDE = H // PATCH           # 8x8 patch grid
NPATCH = NP_SIDE * NP_SIDE
TAPS = 9
# depthwise taps whose accumulation runs on DVE instead of PE identity-matmuls
DVE_TAPS = (0, 2, 6)
BF16 = mybir.dt.bfloat16
F32 = mybir.dt.float32

_CACHED_NC = None


def _split_multiwaits(nc):
    """walrus codegen in this toolchain accepts only ONE embedded sync wait
    per instruction; hoist extras onto preceding NOPs on the same engine."""
    for f in nc.m.functions:
        for blk in f.blocks:
            new_insts = []
            for inst in blk.instructions:
                si = inst.sync_info
                if si is not None and len(si.on_wait) > 1:
                    waits = list(si.on_wait)
                    for w in waits[:-1]:
                        nop = mybir.InstNoOp(
                            name=nc.get_next_instruction_name(), ins=[], outs=[]
                        )
                        nop.engine = inst.engine
                        nop.sync_info = mybir.SyncInfo(on_wait=[w], on_update=[])
                        new_insts.append(nop)
                    inst.sync_info = mybir.SyncInfo(
                        on_wait=[waits[-1]], on_update=list(si.on_update)
                    )
                new_insts.append(inst)
            blk.instructions[:] = new_insts


def _build_nc():
    nc = bass.Bass()
    x = nc.declare_dram_parameter("x", [B_CORE, C, H, W], F32, isOutput=False)
    wblk = nc.declare_dram_parameter("wblk", [128, TAPS, 128], BF16, isOutput=False)
    wbm = nc.declare_dram_parameter(
        "wbm", [128, TAPS, NPATCH, PATCH], BF16, isOutput=False
    )
    wid = nc.declare_dram_parameter("wid", [128, 128], BF16, isOutput=False)
    y = nc.declare_dram_parameter("y", [B_CORE, C, H, W], F32, isOutput=True)

    with TileContext(nc) as tc:
        with (
            tc.tile_pool(name="persist", bufs=1) as persist,
            tc.tile_pool(name="binpp", bufs=1) as binpool,
            tc.tile_pool(name="tmpp", bufs=1) as tmppool,
            tc.tile_pool(name="inp", bufs=3) as inpool,
            tc.tile_pool(name="outp", bufs=6) as outpool,
            tc.tile_pool(name="dwps", bufs=2, space="PSUM") as dwpsum,
            tc.tile_pool(name="cvps", bufs=4, space="PSUM") as cvpsum,
        ):
            wblk_sb = persist.tile([128, TAPS, 128], BF16)
            wb_sb = persist.tile([128, TAPS, NPATCH, PATCH], BF16)
            wid_sb = persist.tile([128, 128], BF16)
            bsa0 = persist.tile([128, H + 2, W + 2], BF16)
            bsa1 = persist.tile([128, H + 2, W + 2], BF16)

            nc.sync.dma_start(out=wid_sb, in_=wid[:])
            nc.sync.dma_start(out=wblk_sb, in_=wblk[:])
            nc.sync.dma_start(out=wb_sb, in_=wbm[:])

            def zero_borders_patch(t):
                # pad ring of each 18x18 patch block; interior rewritten later
                nc.gpsimd.memset(t[:, :, 0, :], 0.0)
                nc.gpsimd.memset(t[:, :, 17, :], 0.0)
                nc.gpsimd.memset(t[:, :, 1:17, 0], 0.0)
                nc.gpsimd.memset(t[:, :, 1:17, 17], 0.0)

            # pre-zero the padded pool tiles once (interiors get rewritten,
            # borders must remain zero forever); zero in first-use order so
            # group 0's tiles clear the GPSIMD queue first
            binp_tiles = [
        binpool.tile([128, NP_SIDE, PATCH + 2, PATCH + 2], BF16, name=f"binp{i}")
        for i in range(4)
            ]
            tmp_tiles = [
        tmppool.tile([128, NP_SIDE, PATCH + 2, PATCH + 2], BF16, name=f"tmp{i}")
        for i in range(6)
            ]
            part_tiles = [
        tmppool.tile([128, NP_SIDE, PATCH + 2, PATCH + 2], BF16, name=f"part{i}")
        for i in range(2)
            ]
            zero_borders_patch(binp_tiles[0])
            for tt in tmp_tiles:
                zero_borders_patch(tt)
            zero_borders_patch(part_tiles[0])
            zero_borders_patch(part_tiles[1])
            for bt in binp_tiles[1:]:
                zero_borders_patch(bt)
            # bsa borders are not needed until the first sign2 (~30us in), so
            # they go last in the GPSIMD queue
            for bs in (bsa0, bsa1):
                nc.gpsimd.memset(bs[:, 0, :], 0.0)
                nc.gpsimd.memset(bs[:, 129, :], 0.0)
                nc.gpsimd.memset(bs[:, 1:129, 0], 0.0)
                nc.gpsimd.memset(bs[:, 1:129, 129], 0.0)
            scr_tile = tmppool.tile(
                [128, NP_SIDE, PATCH, PATCH], BF16, name="scr"
            )

            def conv_group(y_pair, bsa, g):
                # conv output rows 16g..16g+15 in chunks of 4 rows
                for cc in range(4):
                    y0 = 16 * g + 4 * cc
                    pt = cvpsum.tile([128, 4, W], F32, name="cvp")
                    for t in range(TAPS):
                        u, v = divmod(t, 3)
                        nc.tensor.matmul(
                            pt,
                            lhsT=wblk_sb[:, t, :],
                            rhs=bsa[:, y0 + u : y0 + u + 4, v : v + W],
                            start=(t == 0),
                            stop=(t == TAPS - 1),
                        )
                    ot = outpool.tile([128, 4, W], F32, name="ot")
                    nc.scalar.activation(
                        ot, pt, mybir.ActivationFunctionType.Copy
                    )
                    nc.sync.dma_start(
                        out=y_pair[:, :, y0 : y0 + 4, :].rearrange(
                            "b c h w -> (b c) h w"
                        ),
                        in_=ot,
                    )

            bi = 0
            ti = 0
            for pi in range(N_PAIRS):
                x_pair = x[2 * pi : 2 * pi + 2]
                y_pair = y[2 * pi : 2 * pi + 2]
                bsa = bsa0 if pi % 2 == 0 else bsa1

                for g in range(NP_SIDE):
                    # ---- load + sign(input) for patch-row g ----
                    ch = inpool.tile([128, PATCH * W], F32, name="ch")
                    nc.sync.dma_start(
                        out=ch,
                        in_=x_pair[:, :, PATCH * g : PATCH * (g + 1), :].rearrange(
                            "b c h w -> (b c) (h w)"
                        ),
                    )
                    binp = binp_tiles[bi % 4]
                    bi += 1
                    nc.scalar.sign(
                        out=binp[:, :, 1:17, 1:17],
                        in_=ch.rearrange(
                            "n (i pc j) -> n pc i j", pc=NP_SIDE, j=PATCH
                        ),
                    )

                    # ---- depthwise: DVE weighted copies + PE tap-sum ----
                    # DVE_TAPS are pre-summed on DVE (shifted-window mults +
                    # adds into a padded partial tile read back through the
                    # center window); the rest go through identity-matmuls.
                    wbr = lambda t: wb_sb[
                        :, t, 8 * g : 8 * g + 8, None, :
                    ].broadcast_to([128, NP_SIDE, PATCH, PATCH])
                    # taper: pair 1's last groups shift more tap-sums onto
                    # DVE, whose work otherwise ends ~23us before PE's
                    dset = DVE_TAPS
                    if pi == N_PAIRS - 1 and g >= 5:
                        dset = (0, 2, 5, 6, 8)
                    part = part_tiles[(pi * NP_SIDE + g) % 2]
                    scr = scr_tile
                    for i, t in enumerate(dset):
                        u, v = divmod(t, 3)
                        win = binp[:, :, u : u + PATCH, v : v + PATCH]
                        if i == 0:
                            nc.vector.tensor_tensor(
                                out=part[:, :, 1:17, 1:17], in0=win, in1=wbr(t),
                                op=mybir.AluOpType.mult,
                            )
                        else:
                            nc.vector.tensor_tensor(
                                out=scr, in0=win, in1=wbr(t),
                                op=mybir.AluOpType.mult,
                            )
                            nc.vector.tensor_add(
                                out=part[:, :, 1:17, 1:17],
                                in0=part[:, :, 1:17, 1:17],
                                in1=scr,
                            )
                    pe_items = [(t, t) for t in range(TAPS) if t not in dset]
                    if dset:
                        pe_items.append((4, None))  # partial via center window
                    pe_tmps = []
                    for t, tap in pe_items:
                        if tap is not None:
                            tmp = tmp_tiles[ti % 6]
                            ti += 1
                            nc.vector.tensor_tensor(
                                out=tmp[:, :, 1:17, 1:17],
                                in0=binp[:, :, 1:17, 1:17],
                                in1=wbr(tap),
                                op=mybir.AluOpType.mult,
                            )
                        else:
                            tmp = part
                        pe_tmps.append(tmp)
                    # half-group psum tiles so sign2 overlaps the next half's MMs
                    for hg in range(2):
                        pg = dwpsum.tile([128, 4, PATCH, PATCH], F32, name="pg")
                        for idx, (t, tap) in enumerate(pe_items):
                            u, v = divmod(t, 3)
                            tmp = pe_tmps[idx]
                            for pi2 in range(2):
                                pp = 4 * hg + 2 * pi2
                                nc.tensor.matmul(
                                    pg[:, 2 * pi2 : 2 * pi2 + 2],
                                    lhsT=wid_sb,
                                    rhs=tmp[
                                        :, pp : pp + 2, u : u + PATCH, v : v + PATCH
                                    ],
                                    start=(idx == 0),
                                    stop=(idx == len(pe_items) - 1),
                                )
                        # ---- sign(self_attention) -> image-padded bsa ----
                        nc.scalar.sign(
                            out=bsa[
                                :,
                                1 + PATCH * g : 1 + PATCH * (g + 1),
                                1 + 64 * hg : 65 + 64 * hg,
                            ].rearrange("n i (pc j) -> n pc i j", pc=4),
                            in_=pg,
                        )

                    # ---- final conv lags one group (needs bsa row 16g+16) ----
                    if g >= 1:
                        conv_group(y_pair, bsa, g - 1)
                conv_group(y_pair, bsa, NP_SIDE - 1)

    _split_multiwaits(nc)
    return nc


def _host_weights(patch_filters, output_filters):
    # block-diagonal lhsT for the final conv: [tap, k=cin+64*s, m=cout+64*s]
    ofs = np.sign(np.asarray(output_filters, np.float32))          # [o, i, 3, 3]
    oft = ofs.transpose(2, 3, 1, 0).reshape(TAPS, C, C)            # [t, cin, cout]
    wblk = np.zeros((TAPS, 128, 128), np.float32)
    wblk[:, :C, :C] = oft
    wblk[:, C:, C:] = oft
    wblk = wblk.transpose(1, 0, 2)                                 # [k, t, m]
    # depthwise weights broadcast along j: [t, c + 64*s, p, j]
    pfs = np.sign(np.asarray(patch_filters, np.float32))[:, :, 0]  # [c, p, 3, 3]
    wb = pfs.transpose(2, 3, 0, 1).reshape(TAPS, C, NPATCH)        # [t, c, p]
    wbm = np.repeat(wb[:, :, :, None], PATCH, axis=3)              # [t, c, p, j]
    wbm = np.concatenate([wbm, wbm], axis=1)                       # [t, 128, p, j]
    wbm = wbm.transpose(1, 0, 2, 3)                                # [k, t, p, j]
    wid = np.eye(128, dtype=np.float32)
    to_bf = lambda a: np.ascontiguousarray(a).astype(ml_dtypes.bfloat16)
    return to_bf(wblk), to_bf(wbm), to_bf(wid)


def kernel(input, k, t, patch_filters, output_filters):
    global _CACHED_NC
    if _CACHED_NC is None:
        _CACHED_NC = _build_nc()
    nc = _CACHED_NC

    x = np.ascontiguousarray(np.asarray(input, np.float32))
    wblk, wbm, wid = _host_weights(patch_filters, output_filters)
    in_maps = [
        {"x": np.ascontiguousarray(x[i * B_CORE : (i + 1) * B_CORE]),
         "wblk": wblk, "wbm": wbm, "wid": wid}
        for i in range(N_CORES)
    ]
    res = run_bass_kernel_spmd(nc, in_maps, list(range(N_CORES)))
    return np.concatenate([r["y"] for r in res.results], axis=0)
